# revision 27
# baseline (speedup 1.0000x reference)
"""Trainium2 Bass kernel for the 25-step spiking MLP (784 -> 1000 -> 10).

Data-parallel over batch: 4096 rows split across 8 NeuronCores (512 each).

Layer-1 state per group pair is either
  G-coding: G = mem1 - 1, spikes {0,1} fp8, spike-gen = DVE tensor_scalar
            is_gt 0 (2x_2P mode), or
  E-coding: E = 2*(mem1 - 1), spikes +-1 fp8, spike-gen = ACT Sign.
Both use the same per-step ops: STT (state*beta - spk), then a per-element
constant add C (DMA-engine accumulate via SWDGE where routed, else an
engine STT-add), then spike-gen. C magnitudes stay O(1) so fp32 rounding
matches the reference's spike decisions.

fc1 is exact fp32 on the PE (56 matmuls); group chains start as their
cur1 column block lands (wave-pipelined lags), buffering fp8 spikes in a
rotating slot pool so layer-1 runs ahead of fc2.

fc2 packs W2 bf16 hi|lo into M=20 stationary per hidden tile: 8 matmuls
per step streaming the fp8 spikes. PSUM [20,512] -> ACT copy -> 3-step
batch -> 4 PE transposes -> one strided DVE add (hi+lo) -> layer-2 LIF
on [128,40] tiles with scalar threshold; records accumulate in SBUF and
DMA out at the end in a host-transposable layout.
"""

import os
import numpy as np
import ml_dtypes

import concourse.bass as bass
import concourse.mybir as mybir
import concourse.tile as tile
from concourse import bacc
from concourse.bass_utils import run_bass_kernel_spmd

F32 = mybir.dt.float32
BF16 = mybir.dt.bfloat16
F8 = mybir.dt.float8e4
ALU = mybir.AluOpType
ACTF = mybir.ActivationFunctionType
FP8NP = ml_dtypes.float8_e4m3
BF16NP = ml_dtypes.bfloat16

N_CORES = 8
B = 4096
PB = B // N_CORES          # 512 batch rows per core
INP = 784
KA = INP + 1               # ones-row folds the bias in
HID = 1000
HIDP = 1024
OUT = 10
T = 25
BETA = 0.95
BIG = 1.0e6

NG = 8                     # hidden groups of 128
NBT = PB // 128            # 4 batch chunks
KSPLITS = [(i * 128, min(128, KA - i * 128)) for i in range((KA + 127) // 128)]

# ---- schedule / routing config (pairs of adjacent groups) ----
NPAIR = 4
LAG = [3, 6, 9, 12]        # wave lag per pair (pair p = groups 2p, 2p+1)
SLOTS = LAG[-1] + 4        # spike slot rotation depth
# coding per pair: 'G' ({0,1} spikes, DVE TS spike) | 'E' (+-1, ACT Sign)
CODING = ['E', 'E', 'E', 'E']
# STT engine per pair: 'dve' only (scalar_tensor_tensor is DVE-only)
STT_ENG = ['dve', 'dve', 'dve', 'dve']
# C-add route per pair: 'dma' | 'dve' | 'pool'
CADD = ['dma', 'dma', 'dma', 'pool']
WMAX = T + LAG[-1]
DBG_WMAX = int(os.environ.get("K_MAXW", WMAX))
DBG_L2STAGE = int(os.environ.get("K_L2STAGE", 4))
DBG_DUMP = int(os.environ.get("K_DUMP", 0))


def _build_program():
    nc = bacc.Bacc("TRN2", target_bir_lowering=False, debug=False,
                   enable_partition_id=False)

    xt_d = nc.dram_tensor("xt", [KA, PB], F32, kind="ExternalInput")
    w1t_d = nc.dram_tensor("w1t", [KA, HIDP], F32, kind="ExternalInput")
    w2p_d = nc.dram_tensor("w2p", [HIDP, 20], BF16, kind="ExternalInput")
    idt_d = nc.dram_tensor("idt", [20, 20], F32, kind="ExternalInput")
    dC_d = nc.dram_tensor("dC", [128, NG * PB], F32, kind="ExternalOutput")
    dG_d = nc.dram_tensor("dG", [128, NG * PB], F32, kind="ExternalOutput")
    dS_d = nc.dram_tensor("dS", [128, NG * PB], F8, kind="ExternalOutput")
    ospk_d = nc.dram_tensor("ospk", [128, T * NBT * OUT], F32,
                            kind="ExternalOutput")
    omem_d = nc.dram_tensor("omem", [128, T * NBT * OUT], F32,
                            kind="ExternalOutput")

    with tile.TileContext(nc) as tc:
        with (
            tc.tile_pool(name="state", bufs=1) as state,
            tc.tile_pool(name="spkp", bufs=SLOTS) as spkp,
            tc.tile_pool(name="csp", bufs=2) as csp,
            tc.tile_pool(name="fc1ps", bufs=2, space="PSUM") as fc1ps,
            tc.tile_pool(name="c2ps", bufs=2, space="PSUM") as c2ps,
            tc.tile_pool(name="tps", bufs=1, space="PSUM") as tps,
        ):
            # ---- load inputs ----
            xt_t, w1t_t = [], []
            for i, (k0, kk) in enumerate(KSPLITS):
                xk = state.tile([kk, PB], F32, tag=f"xt{i}")
                nc.sync.dma_start(xk[:], xt_d.ap()[k0:k0 + kk, :])
                xt_t.append(xk)
                wk = state.tile([kk, HIDP], F32, tag=f"w1t{i}")
                nc.sync.dma_start(wk[:], w1t_d.ap()[k0:k0 + kk, :])
                w1t_t.append(wk)
            w2_t = []
            for j in range(NG):
                wj = state.tile([128, 20], BF16, tag=f"w2p{j}")
                nc.sync.dma_start(wj[:], w2p_d.ap()[j * 128:(j + 1) * 128, :])
                w2_t.append(wj)
            # ---- persistent state ----
            G = state.tile([128, NG * PB], F32, tag="G")
            C = state.tile([128, NG * PB], F32, tag="C")
            idt = state.tile([20, 20], F32, tag="idt")
            nc.sync.dma_start(idt[:], idt_d.ap())
            rec_spk = state.tile([128, T * NBT * OUT], F32, tag="rspk")
            rec_mem = state.tile([128, T * NBT * OUT], F32, tag="rmem")
            z40 = state.tile([128, NBT * OUT], F32, tag="z40")
            l2u = state.tile([128, NBT * OUT], F32, tag="l2u")
            l2v = state.tile([128, NBT * OUT], F32, tag="l2v")
            nc.vector.memset(z40[:], 0.0)
            bG = state.tile([128, 1], F32, tag="bG")
            nc.vector.memset(bG[:], -(1.0 - BETA))
            bE = state.tile([128, 1], F32, tag="bE")
            nc.vector.memset(bE[:], -(2.0 * (1.0 - BETA) + 1.0))

            def pcols(p):
                return slice(2 * p * PB, (2 * p + 2) * PB)

            # state init per pair coding
            for p in range(NPAIR):
                init = -1.0 if CODING[p] == 'G' else -2.0
                nc.vector.memset(G[:, pcols(p)], init)

            # spike slot tiles (rotation); slot for t is spk_slots[t % SLOTS]
            spk_slots = {}

            def new_slot():
                return spkp.tile([128, NG * PB], F8, tag="spk", name="spk")

            s0 = new_slot()
            for p in range(NPAIR):
                sp0 = 0.0 if CODING[p] == 'G' else -1.0
                nc.vector.memset(s0[:, pcols(p)], sp0)
            spk_slots[0] = s0

            # ---- fc1 (fp32, exact) + C generation ----
            for g in range(NG):
                ps = fc1ps.tile([128, PB], F32, tag="fc1")
                for i, (k0, kk) in enumerate(KSPLITS):
                    nc.tensor.matmul(
                        ps[:], w1t_t[i][:, g * 128:(g + 1) * 128], xt_t[i][:],
                        start=(i == 0), stop=(i == len(KSPLITS) - 1))
                p = g // 2
                if CODING[p] == 'G':
                    # C = cur1 - (1-beta)
                    nc.scalar.activation(C[:, g * PB:(g + 1) * PB], ps[:],
                                         ACTF.Identity, bias=bG[:])
                else:
                    # C = 2*cur1 - 2*(1-beta) - 1
                    nc.scalar.activation(C[:, g * PB:(g + 1) * PB], ps[:],
                                         ACTF.Identity, bias=bE[:],
                                         scale=2.0)

            # ---- wave-pipelined layer-1 + fc2 + layer-2 ----
            def l1_ops(w):
                # returns list of (pair, t) active this wave
                act = []
                for p in range(NPAIR):
                    t = w - LAG[p]
                    if 1 <= t <= T:
                        act.append((p, t))
                return act

            for w in range(1, min(WMAX, DBG_WMAX) + 1):
                act = l1_ops(w)
                if not act:
                    continue
                # allocate this wave's spike-slot tiles for steps being made
                for p, t in act:
                    if t not in spk_slots:
                        spk_slots[t] = new_slot()

                # 1) STT: u = beta*G - spk_prev  (in place on G)
                for p, t in act:
                    sl = pcols(p)
                    nc.vector.scalar_tensor_tensor(G[:, sl], G[:, sl], BETA,
                                                   spk_slots[t - 1][:, sl],
                                                   ALU.mult, ALU.subtract)

                # 2) C add: dma pairs merged into one transfer when adjacent
                # merged accumulate runs, capped at 2048 cols (HW limit:
                # wider SWDGE accumulate transfers corrupt data)
                dma_pairs = [p for p, t in act if CADD[p] == 'dma']
                i = 0
                while i < len(dma_pairs):
                    j = i
                    while (j + 1 < len(dma_pairs)
                           and dma_pairs[j + 1] == dma_pairs[j] + 1
                           and j - i < 1):
                        j += 1
                    lo = 2 * dma_pairs[i] * PB
                    hi = (2 * dma_pairs[j] + 2) * PB
                    nc.gpsimd.dma_start(G[:, lo:hi], C[:, lo:hi],
                                        accum_op=ALU.add)
                    i = j + 1
                for p, t in act:
                    if CADD[p] in ('dve', 'pool'):
                        sl = pcols(p)
                        eng = nc.vector if CADD[p] == 'dve' else nc.gpsimd
                        eng.tensor_tensor(G[:, sl], G[:, sl], C[:, sl],
                                          ALU.add)

                # 3) spike-gen into this step's slot
                for p, t in act:
                    sl = pcols(p)
                    dst = spk_slots[t][:, sl]
                    if CODING[p] == 'G':
                        nc.vector.tensor_scalar(dst, G[:, sl], 0.0, None,
                                                ALU.is_gt)
                    else:
                        nc.scalar.activation(dst, G[:, sl], ACTF.Sign)

                # 4) fc2 for the step all pairs have finished
                tq = w - LAG[-1]
                if 1 <= tq <= T:
                    c2p = c2ps.tile([20, PB], F32, tag="c2p")
                    s = spk_slots[tq]
                    for j in range(NG):
                        nc.tensor.matmul(c2p[:], w2_t[j][:],
                                         s[:, j * PB:(j + 1) * PB],
                                         start=(j == 0), stop=(j == NG - 1))
                    cs = csp.tile([20, PB], F32, tag="cs", name="cs")
                    nc.scalar.copy(cs[:], c2p[:])
                    # PE transposes into 4 bank-aligned psum columns
                    tp = tps.tile([128, 4 * 512], F32, tag="tp", name="tp")
                    for bt in range(NBT):
                        nc.tensor.matmul(tp[:, bt * 512:bt * 512 + 20],
                                         cs[:, bt * 128:(bt + 1) * 128],
                                         idt[:], start=True, stop=True,
                                         is_transpose=True)
                    tp3 = tp[:].rearrange("p (b x) -> p b x", x=512)
                    hi = tp3[:, :, 0:OUT]
                    lo = tp3[:, :, OUT:2 * OUT]
                    # layer-2 LIF on [128, 40]
                    cur = slice((tq - 1) * NBT * OUT, tq * NBT * OUT)
                    if tq == 1:
                        pm, psk = z40[:], z40[:]
                    else:
                        prv = slice((tq - 2) * NBT * OUT,
                                    (tq - 1) * NBT * OUT)
                        pm, psk = rec_mem[:, prv], rec_spk[:, prv]
                    nc.vector.scalar_tensor_tensor(
                        l2u[:], pm, BETA, psk, ALU.mult, ALU.subtract)
                    nc.vector.tensor_tensor(
                        l2v[:].rearrange("p (b o) -> p b o", o=OUT),
                        l2u[:].rearrange("p (b o) -> p b o", o=OUT),
                        hi, ALU.add)
                    nc.vector.tensor_tensor(
                        rec_mem[:, cur].rearrange("p (b o) -> p b o", o=OUT),
                        l2v[:].rearrange("p (b o) -> p b o", o=OUT),
                        lo, ALU.add)
                    nc.vector.tensor_scalar(rec_spk[:, cur],
                                            rec_mem[:, cur], 1.0,
                                            None, ALU.is_gt)

            nc.sync.dma_start(ospk_d.ap(), rec_spk[:])
            nc.sync.dma_start(omem_d.ap(), rec_mem[:])
            if DBG_DUMP:
                nc.sync.dma_start(dC_d.ap(), C[:])
                nc.sync.dma_start(dG_d.ap(), G[:])
                nc.sync.dma_start(dS_d.ap(),
                                  spk_slots[DBG_DUMP][:])

    nc.compile()
    return nc


_NC_CACHE = None


def _prep(x, W1, b1, W2, b2):
    # fc1: w1t = [W1.T; b1-row], padded to 1024 hidden
    w1t = np.zeros((KA, HIDP), np.float32)
    w1t[:INP, :HID] = W1.T
    w1t[INP, :HID] = b1
    w1t[INP, HID] = BIG          # unit 1000: always fires (bias carrier)
    w1t[INP, HID + 1:] = -BIG    # other pads: never fire

    # fc2 rows by group coding
    R = np.zeros((HIDP, OUT), np.float64)
    W2T = W2.T.astype(np.float64)
    half_sum = np.zeros(OUT, np.float64)
    for g in range(NG):
        p = g // 2
        rows = np.arange(g * 128, (g + 1) * 128)
        rows = rows[rows < HID]
        if CODING[p] == 'G':
            R[rows] = W2T[rows]
        else:
            R[rows] = 0.5 * W2T[rows]
            half_sum += 0.5 * W2T[rows].sum(axis=0)
    R[HID] = b2.astype(np.float64) + half_sum
    hi = R.astype(BF16NP)
    lo = (R - hi.astype(np.float64)).astype(BF16NP)
    w2p = np.zeros((HIDP, 20), BF16NP)
    w2p[:, 0:OUT] = hi
    w2p[:, OUT:2 * OUT] = lo

    xt = np.concatenate([x.T, np.ones((1, x.shape[0]), np.float32)], axis=0)
    return w1t, w2p, xt


def kernel(x, W1, b1, W2, b2):
    global _NC_CACHE
    x = np.ascontiguousarray(np.asarray(x, np.float32))
    W1 = np.asarray(W1, np.float32)
    b1 = np.asarray(b1, np.float32)
    W2 = np.asarray(W2, np.float32)
    b2 = np.asarray(b2, np.float32)

    w1t, w2p, xt = _prep(x, W1, b1, W2, b2)

    if _NC_CACHE is None:
        _NC_CACHE = _build_program()
    nc = _NC_CACHE

    in_maps = []
    for c in range(N_CORES):
        sl = slice(c * PB, (c + 1) * PB)
        in_maps.append({
            "xt": np.ascontiguousarray(xt[:, sl]),
            "w1t": w1t,
            "w2p": w2p,
            "idt": np.eye(20, dtype=np.float32),
        })

    res = run_bass_kernel_spmd(nc, in_maps, core_ids=list(range(N_CORES)))
    kernel.last_results = res

    ospk = np.empty((T, B, OUT), np.float32)
    omem = np.empty((T, B, OUT), np.float32)
    # rec[p, t*40 + bt*10 + o] -> [t, 128*bt+p, o]
    for c in range(N_CORES):
        sl = slice(c * PB, (c + 1) * PB)
        a = res.results[c]["ospk"].reshape(128, T, NBT, OUT)
        ospk[:, sl, :] = a.transpose(1, 2, 0, 3).reshape(T, PB, OUT)
        a = res.results[c]["omem"].reshape(128, T, NBT, OUT)
        omem[:, sl, :] = a.transpose(1, 2, 0, 3).reshape(T, PB, OUT)
    return ospk, omem


# revision 30
# speedup vs baseline: 1.0931x; 1.0931x over previous
"""Trainium2 Bass kernel for the 25-step spiking MLP (784 -> 1000 -> 10).

Data-parallel over batch: 4096 rows split across 8 NeuronCores (512 each).

Layer-1 state per group pair is either
  G-coding: G = mem1 - 1, spikes {0,1} fp8, spike-gen = DVE tensor_scalar
            is_gt 0 (2x_2P mode), or
  E-coding: E = 2*(mem1 - 1), spikes +-1 fp8, spike-gen = ACT Sign.
Both use the same per-step ops: STT (state*beta - spk), then a per-element
constant add C (DMA-engine accumulate via SWDGE where routed, else an
engine STT-add), then spike-gen. C magnitudes stay O(1) so fp32 rounding
matches the reference's spike decisions.

fc1 is exact fp32 on the PE (56 matmuls); group chains start as their
cur1 column block lands (wave-pipelined lags), buffering fp8 spikes in a
rotating slot pool so layer-1 runs ahead of fc2.

fc2 packs W2 bf16 hi|lo into M=20 stationary per hidden tile: 8 matmuls
per step streaming the fp8 spikes. PSUM [20,512] -> ACT copy -> 3-step
batch -> 4 PE transposes -> one strided DVE add (hi+lo) -> layer-2 LIF
on [128,40] tiles with scalar threshold; records accumulate in SBUF and
DMA out at the end in a host-transposable layout.
"""

import os
import numpy as np
import ml_dtypes

import concourse.bass as bass
import concourse.mybir as mybir
import concourse.tile as tile
from concourse import bacc
from concourse.bass_utils import run_bass_kernel_spmd

F32 = mybir.dt.float32
BF16 = mybir.dt.bfloat16
F8 = mybir.dt.float8e4
ALU = mybir.AluOpType
ACTF = mybir.ActivationFunctionType
FP8NP = ml_dtypes.float8_e4m3
BF16NP = ml_dtypes.bfloat16

N_CORES = 8
B = 4096
PB = B // N_CORES          # 512 batch rows per core
INP = 784
KA = INP + 1               # ones-row folds the bias in
HID = 1000
HIDP = 1024
OUT = 10
T = 25
BETA = 0.95
BIG = 1.0e6

NG = 8                     # hidden groups of 128
NBT = PB // 128            # 4 batch chunks
KSPLITS = [(i * 128, min(128, KA - i * 128)) for i in range((KA + 127) // 128)]

# ---- schedule / routing config (pairs of adjacent groups) ----
NPAIR = 4
LAG = [3, 6, 9, 12]        # wave lag per pair (pair p = groups 2p, 2p+1)
SLOTS = 16                 # spike slot depth (run-ahead during fc1 phase)
# C-add route per pair: pairs 0,1 share one merged accumulate DMA;
# pairs 2,3 use Pool tensor_tensor adds
CADD = ['dma', 'dma', 'pool', 'pool']
WMAX = T + LAG[-1]
DBG_WMAX = int(os.environ.get("K_MAXW", WMAX))
DBG_L2STAGE = int(os.environ.get("K_L2STAGE", 4))
DBG_DUMP = int(os.environ.get("K_DUMP", 0))


def _build_program():
    nc = bacc.Bacc("TRN2", target_bir_lowering=False, debug=False,
                   enable_partition_id=False)

    xt_d = nc.dram_tensor("xt", [KA, PB], F32, kind="ExternalInput")
    w1t_d = nc.dram_tensor("w1t", [KA, HIDP], F32, kind="ExternalInput")
    w2p_d = nc.dram_tensor("w2p", [HIDP, 20], BF16, kind="ExternalInput")
    idt_d = nc.dram_tensor("idt", [20, 20], F32, kind="ExternalInput")
    dC_d = nc.dram_tensor("dC", [128, NG * PB], F32, kind="ExternalOutput")
    dG_d = nc.dram_tensor("dG", [128, NG * PB], F32, kind="ExternalOutput")
    dS_d = nc.dram_tensor("dS", [128, NG * PB], F8, kind="ExternalOutput")
    ospk_d = nc.dram_tensor("ospk", [128, T * NBT * OUT], F32,
                            kind="ExternalOutput")
    omem_d = nc.dram_tensor("omem", [128, T * NBT * OUT], F32,
                            kind="ExternalOutput")

    with tile.TileContext(nc) as tc:
        with (
            tc.tile_pool(name="state", bufs=1) as state,
            tc.tile_pool(name="spkp", bufs=SLOTS) as spkp,
            tc.tile_pool(name="csp", bufs=2) as csp,
            tc.tile_pool(name="fc1ps", bufs=2, space="PSUM") as fc1ps,
            tc.tile_pool(name="c2ps", bufs=2, space="PSUM") as c2ps,
            tc.tile_pool(name="tps", bufs=1, space="PSUM") as tps,
        ):
            # ---- load inputs ----
            xt_t, w1t_t = [], []
            for i, (k0, kk) in enumerate(KSPLITS):
                xk = state.tile([kk, PB], F32, tag=f"xt{i}")
                nc.sync.dma_start(xk[:], xt_d.ap()[k0:k0 + kk, :])
                xt_t.append(xk)
                wk = state.tile([kk, HIDP], F32, tag=f"w1t{i}")
                nc.sync.dma_start(wk[:], w1t_d.ap()[k0:k0 + kk, :])
                w1t_t.append(wk)
            w2_t = []
            for j in range(NG):
                wj = state.tile([128, 20], BF16, tag=f"w2p{j}")
                nc.sync.dma_start(wj[:], w2p_d.ap()[j * 128:(j + 1) * 128, :])
                w2_t.append(wj)
            # ---- persistent state ----
            G = state.tile([128, NG * PB], F32, tag="G")
            C = state.tile([128, NG * PB], F32, tag="C")
            idt = state.tile([20, 20], F32, tag="idt")
            nc.sync.dma_start(idt[:], idt_d.ap())
            rec_spk = state.tile([128, T * NBT * OUT], F32, tag="rspk")
            rec_mem = state.tile([128, T * NBT * OUT], F32, tag="rmem")
            z40 = state.tile([128, NBT * OUT], F32, tag="z40")
            l2u = state.tile([128, NBT * OUT], F32, tag="l2u")
            l2v = state.tile([128, NBT * OUT], F32, tag="l2v")
            nc.vector.memset(z40[:], 0.0)
            bE = state.tile([128, 1], F32, tag="bE")
            nc.vector.memset(bE[:], -(2.0 * (1.0 - BETA) + 1.0))

            def pcols(p):
                return slice(2 * p * PB, (2 * p + 2) * PB)

            # E_0 = 2*(mem_0 - 1) = -2
            nc.vector.memset(G[:], -2.0)

            # spike slot tiles (rotation); slot for t is spk_slots[t % SLOTS]
            spk_slots = {}

            def new_slot():
                return spkp.tile([128, NG * PB], F8, tag="spk", name="spk")

            s0 = new_slot()
            nc.vector.memset(s0[:], -1.0)
            spk_slots[0] = s0

            # ---- fc1 (fp32, exact) + C generation ----
            for g in range(NG):
                ps = fc1ps.tile([128, PB], F32, tag="fc1")
                for i, (k0, kk) in enumerate(KSPLITS):
                    nc.tensor.matmul(
                        ps[:], w1t_t[i][:, g * 128:(g + 1) * 128], xt_t[i][:],
                        start=(i == 0), stop=(i == len(KSPLITS) - 1))
                gsl = slice(g * PB, (g + 1) * PB)
                # C = 2*cur1 - 2*(1-beta) - 1 (E-coding constant)
                nc.scalar.activation(C[:, gsl], ps[:], ACTF.Identity,
                                     bias=bE[:], scale=2.0)

            # ---- wave-pipelined layer-1 + fc2 + layer-2 ----
            def l1_ops(w):
                # returns list of (pair, t) active this wave
                act = []
                for p in range(NPAIR):
                    t = w - LAG[p]
                    if 1 <= t <= T:
                        act.append((p, t))
                return act

            for w in range(1, min(WMAX, DBG_WMAX) + 1):
                act = l1_ops(w)
                if not act:
                    continue
                # allocate this wave's spike-slot tiles for steps being made
                for p, t in act:
                    if t not in spk_slots:
                        spk_slots[t] = new_slot()

                # 1) STT: E = beta*E - sgn_prev  (in place on G tile)
                for p, t in act:
                    sl = pcols(p)
                    nc.vector.scalar_tensor_tensor(G[:, sl], G[:, sl], BETA,
                                                   spk_slots[t - 1][:, sl],
                                                   ALU.mult, ALU.subtract)

                # 2) C-add: merged accumulate DMA for adjacent dma pairs
                #    (<=2048 cols per transfer), Pool TT add otherwise
                dma_pairs = [p for p, t in act if CADD[p] == 'dma']
                i = 0
                while i < len(dma_pairs):
                    j = i
                    while (j + 1 < len(dma_pairs)
                           and dma_pairs[j + 1] == dma_pairs[j] + 1
                           and j - i < 1):
                        j += 1
                    lo = 2 * dma_pairs[i] * PB
                    hi = (2 * dma_pairs[j] + 2) * PB
                    nc.gpsimd.dma_start(G[:, lo:hi], C[:, lo:hi],
                                        accum_op=ALU.add)
                    i = j + 1
                for p, t in act:
                    if CADD[p] == 'pool':
                        sl = pcols(p)
                        nc.gpsimd.tensor_tensor(G[:, sl], G[:, sl],
                                                C[:, sl], ALU.add)

                # 3) spike-gen: sgn = Sign(E) in +-1 fp8 (ACT)
                for p, t in act:
                    sl = pcols(p)
                    nc.scalar.activation(spk_slots[t][:, sl], G[:, sl],
                                         ACTF.Sign)

                # 4) fc2 for the step all pairs have finished
                tq = w - LAG[-1]
                if 1 <= tq <= T:
                    c2p = c2ps.tile([20, PB], F32, tag="c2p")
                    s = spk_slots[tq]
                    for j in range(NG):
                        nc.tensor.matmul(c2p[:], w2_t[j][:],
                                         s[:, j * PB:(j + 1) * PB],
                                         start=(j == 0), stop=(j == NG - 1))
                    cs = csp.tile([20, PB], F32, tag="cs", name="cs")
                    nc.scalar.copy(cs[:], c2p[:])
                    # PE transposes into 4 bank-aligned psum columns
                    tp = tps.tile([128, 4 * 512], F32, tag="tp", name="tp")
                    for bt in range(NBT):
                        nc.tensor.matmul(tp[:, bt * 512:bt * 512 + 20],
                                         cs[:, bt * 128:(bt + 1) * 128],
                                         idt[:], start=True, stop=True,
                                         is_transpose=True)
                    tp3 = tp[:].rearrange("p (b x) -> p b x", x=512)
                    hi = tp3[:, :, 0:OUT]
                    lo = tp3[:, :, OUT:2 * OUT]
                    # layer-2 LIF on [128, 40]
                    cur = slice((tq - 1) * NBT * OUT, tq * NBT * OUT)
                    if tq == 1:
                        pm, psk = z40[:], z40[:]
                    else:
                        prv = slice((tq - 2) * NBT * OUT,
                                    (tq - 1) * NBT * OUT)
                        pm, psk = rec_mem[:, prv], rec_spk[:, prv]
                    nc.vector.scalar_tensor_tensor(
                        l2u[:], pm, BETA, psk, ALU.mult, ALU.subtract)
                    nc.vector.tensor_tensor(
                        l2v[:].rearrange("p (b o) -> p b o", o=OUT),
                        l2u[:].rearrange("p (b o) -> p b o", o=OUT),
                        hi, ALU.add)
                    nc.vector.tensor_tensor(
                        rec_mem[:, cur].rearrange("p (b o) -> p b o", o=OUT),
                        l2v[:].rearrange("p (b o) -> p b o", o=OUT),
                        lo, ALU.add)
                    nc.vector.tensor_scalar(rec_spk[:, cur],
                                            rec_mem[:, cur], 1.0,
                                            None, ALU.is_gt)

            nc.sync.dma_start(ospk_d.ap(), rec_spk[:])
            nc.sync.dma_start(omem_d.ap(), rec_mem[:])
            if DBG_DUMP:
                nc.sync.dma_start(dC_d.ap(), C[:])
                nc.sync.dma_start(dG_d.ap(), G[:])
                nc.sync.dma_start(dS_d.ap(),
                                  spk_slots[DBG_DUMP][:])

    nc.compile()
    return nc


_NC_CACHE = None


def _prep(x, W1, b1, W2, b2):
    # fc1: w1t = [W1.T; b1-row], padded to 1024 hidden
    w1t = np.zeros((KA, HIDP), np.float32)
    w1t[:INP, :HID] = W1.T
    w1t[INP, :HID] = b1
    w1t[INP, HID] = BIG          # unit 1000: always fires (bias carrier)
    w1t[INP, HID + 1:] = -BIG    # other pads: never fire

    # fc2 rows (+-1 sgn everywhere: rows W2T/2, bias row b2 + sum(W2T)/2)
    R = np.zeros((HIDP, OUT), np.float64)
    W2T = W2.T.astype(np.float64)
    R[:HID] = 0.5 * W2T
    R[HID] = b2.astype(np.float64) + 0.5 * W2T.sum(axis=0)
    hi = R.astype(BF16NP)
    lo = (R - hi.astype(np.float64)).astype(BF16NP)
    w2p = np.zeros((HIDP, 20), BF16NP)
    w2p[:, 0:OUT] = hi
    w2p[:, OUT:2 * OUT] = lo

    xt = np.concatenate([x.T, np.ones((1, x.shape[0]), np.float32)], axis=0)
    return w1t, w2p, xt


def kernel(x, W1, b1, W2, b2):
    global _NC_CACHE
    x = np.ascontiguousarray(np.asarray(x, np.float32))
    W1 = np.asarray(W1, np.float32)
    b1 = np.asarray(b1, np.float32)
    W2 = np.asarray(W2, np.float32)
    b2 = np.asarray(b2, np.float32)

    w1t, w2p, xt = _prep(x, W1, b1, W2, b2)

    if _NC_CACHE is None:
        _NC_CACHE = _build_program()
    nc = _NC_CACHE

    in_maps = []
    for c in range(N_CORES):
        sl = slice(c * PB, (c + 1) * PB)
        in_maps.append({
            "xt": np.ascontiguousarray(xt[:, sl]),
            "w1t": w1t,
            "w2p": w2p,
            "idt": np.eye(20, dtype=np.float32),
        })

    res = run_bass_kernel_spmd(nc, in_maps, core_ids=list(range(N_CORES)))
    kernel.last_results = res

    ospk = np.empty((T, B, OUT), np.float32)
    omem = np.empty((T, B, OUT), np.float32)
    # rec[p, t*40 + bt*10 + o] -> [t, 128*bt+p, o]
    for c in range(N_CORES):
        sl = slice(c * PB, (c + 1) * PB)
        a = res.results[c]["ospk"].reshape(128, T, NBT, OUT)
        ospk[:, sl, :] = a.transpose(1, 2, 0, 3).reshape(T, PB, OUT)
        a = res.results[c]["omem"].reshape(128, T, NBT, OUT)
        omem[:, sl, :] = a.transpose(1, 2, 0, 3).reshape(T, PB, OUT)
    return ospk, omem


# revision 31
# speedup vs baseline: 1.2899x; 1.1800x over previous
"""Trainium2 Bass kernel for the 25-step spiking MLP (784 -> 1000 -> 10).

Data-parallel over batch: 4096 rows split across 8 NeuronCores (512 each).

Layer-1 state per group pair is either
  G-coding: G = mem1 - 1, spikes {0,1} fp8, spike-gen = DVE tensor_scalar
            is_gt 0 (2x_2P mode), or
  E-coding: E = 2*(mem1 - 1), spikes +-1 fp8, spike-gen = ACT Sign.
Both use the same per-step ops: STT (state*beta - spk), then a per-element
constant add C (DMA-engine accumulate via SWDGE where routed, else an
engine STT-add), then spike-gen. C magnitudes stay O(1) so fp32 rounding
matches the reference's spike decisions.

fc1 is exact fp32 on the PE (56 matmuls); group chains start as their
cur1 column block lands (wave-pipelined lags), buffering fp8 spikes in a
rotating slot pool so layer-1 runs ahead of fc2.

fc2 packs W2 bf16 hi|lo into M=20 stationary per hidden tile: 8 matmuls
per step streaming the fp8 spikes. PSUM [20,512] -> ACT copy -> 3-step
batch -> 4 PE transposes -> one strided DVE add (hi+lo) -> layer-2 LIF
on [128,40] tiles with scalar threshold; records accumulate in SBUF and
DMA out at the end in a host-transposable layout.
"""

import os
import numpy as np
import ml_dtypes

import concourse.bass as bass
import concourse.mybir as mybir
import concourse.tile as tile
from concourse import bacc
from concourse.bass_utils import run_bass_kernel_spmd

F32 = mybir.dt.float32
BF16 = mybir.dt.bfloat16
F8 = mybir.dt.float8e4
ALU = mybir.AluOpType
ACTF = mybir.ActivationFunctionType
FP8NP = ml_dtypes.float8_e4m3
BF16NP = ml_dtypes.bfloat16

N_CORES = 8
B = 4096
PB = B // N_CORES          # 512 batch rows per core
INP = 784
KA = INP + 1               # ones-row folds the bias in
HID = 1000
HIDP = 1024
OUT = 10
T = 25
BETA = 0.95
BIG = 1.0e6

NG = 8                     # hidden groups of 128
NBT = PB // 128            # 4 batch chunks
KSPLITS = [(i * 128, min(128, KA - i * 128)) for i in range((KA + 127) // 128)]

# ---- schedule / routing config (pairs of adjacent groups) ----
NPAIR = 4
LAG = [3, 6, 9, 12]        # wave lag per pair (pair p = groups 2p, 2p+1)
SLOTS = 16                 # spike slot depth (run-ahead during fc1 phase)
# C-add route per pair (engine adds only; DMA accumulate latency was
# gating the chains)
CADD = ['dve', 'pool', 'pool', 'pool']
WMAX = T + LAG[-1]
DBG_WMAX = int(os.environ.get("K_MAXW", WMAX))
DBG_L2STAGE = int(os.environ.get("K_L2STAGE", 4))
DBG_DUMP = int(os.environ.get("K_DUMP", 0))


def _build_program():
    nc = bacc.Bacc("TRN2", target_bir_lowering=False, debug=False,
                   enable_partition_id=False)

    xt_d = nc.dram_tensor("xt", [KA, PB], F32, kind="ExternalInput")
    w1t_d = nc.dram_tensor("w1t", [KA, HIDP], F32, kind="ExternalInput")
    w2p_d = nc.dram_tensor("w2p", [HIDP, 20], BF16, kind="ExternalInput")
    idt_d = nc.dram_tensor("idt", [20, 20], F32, kind="ExternalInput")
    dC_d = nc.dram_tensor("dC", [128, NG * PB], F32, kind="ExternalOutput")
    dG_d = nc.dram_tensor("dG", [128, NG * PB], F32, kind="ExternalOutput")
    dS_d = nc.dram_tensor("dS", [128, NG * PB], F8, kind="ExternalOutput")
    ospk_d = nc.dram_tensor("ospk", [128, T * NBT * OUT], F32,
                            kind="ExternalOutput")
    omem_d = nc.dram_tensor("omem", [128, T * NBT * OUT], F32,
                            kind="ExternalOutput")

    with tile.TileContext(nc) as tc:
        with (
            tc.tile_pool(name="state", bufs=1) as state,
            tc.tile_pool(name="spkp", bufs=SLOTS) as spkp,
            tc.tile_pool(name="csp", bufs=2) as csp,
            tc.tile_pool(name="fc1ps", bufs=2, space="PSUM") as fc1ps,
            tc.tile_pool(name="c2ps", bufs=2, space="PSUM") as c2ps,
            tc.tile_pool(name="tps", bufs=1, space="PSUM") as tps,
        ):
            # ---- load inputs ----
            xt_t, w1t_t = [], []
            for i, (k0, kk) in enumerate(KSPLITS):
                xk = state.tile([kk, PB], F32, tag=f"xt{i}")
                nc.sync.dma_start(xk[:], xt_d.ap()[k0:k0 + kk, :])
                xt_t.append(xk)
                wk = state.tile([kk, HIDP], F32, tag=f"w1t{i}")
                nc.sync.dma_start(wk[:], w1t_d.ap()[k0:k0 + kk, :])
                w1t_t.append(wk)
            w2_t = []
            for j in range(NG):
                wj = state.tile([128, 20], BF16, tag=f"w2p{j}")
                nc.sync.dma_start(wj[:], w2p_d.ap()[j * 128:(j + 1) * 128, :])
                w2_t.append(wj)
            # ---- persistent state ----
            G = state.tile([128, NG * PB], F32, tag="G")
            C = state.tile([128, NG * PB], F32, tag="C")
            idt = state.tile([20, 20], F32, tag="idt")
            nc.sync.dma_start(idt[:], idt_d.ap())
            rec_spk = state.tile([128, T * NBT * OUT], F32, tag="rspk")
            rec_mem = state.tile([128, T * NBT * OUT], F32, tag="rmem")
            z40 = state.tile([128, NBT * OUT], F32, tag="z40")
            l2u = state.tile([128, NBT * OUT], F32, tag="l2u")
            l2v = state.tile([128, NBT * OUT], F32, tag="l2v")
            nc.vector.memset(z40[:], 0.0)
            bE = state.tile([128, 1], F32, tag="bE")
            nc.vector.memset(bE[:], -(2.0 * (1.0 - BETA) + 1.0))

            def pcols(p):
                return slice(2 * p * PB, (2 * p + 2) * PB)

            # E_0 = 2*(mem_0 - 1) = -2
            nc.vector.memset(G[:], -2.0)

            # spike slot tiles (rotation); slot for t is spk_slots[t % SLOTS]
            spk_slots = {}

            def new_slot():
                return spkp.tile([128, NG * PB], F8, tag="spk", name="spk")

            s0 = new_slot()
            nc.vector.memset(s0[:], -1.0)
            spk_slots[0] = s0

            # ---- fc1 (fp32, exact) + C generation ----
            for g in range(NG):
                ps = fc1ps.tile([128, PB], F32, tag="fc1")
                for i, (k0, kk) in enumerate(KSPLITS):
                    nc.tensor.matmul(
                        ps[:], w1t_t[i][:, g * 128:(g + 1) * 128], xt_t[i][:],
                        start=(i == 0), stop=(i == len(KSPLITS) - 1))
                gsl = slice(g * PB, (g + 1) * PB)
                # C = 2*cur1 - 2*(1-beta) - 1 (E-coding constant)
                nc.scalar.activation(C[:, gsl], ps[:], ACTF.Identity,
                                     bias=bE[:], scale=2.0)

            # ---- wave-pipelined layer-1 + fc2 + layer-2 ----
            def l1_ops(w):
                # returns list of (pair, t) active this wave
                act = []
                for p in range(NPAIR):
                    t = w - LAG[p]
                    if 1 <= t <= T:
                        act.append((p, t))
                return act

            for w in range(1, min(WMAX, DBG_WMAX) + 1):
                act = l1_ops(w)
                if not act:
                    continue
                # allocate this wave's spike-slot tiles for steps being made
                for p, t in act:
                    if t not in spk_slots:
                        spk_slots[t] = new_slot()

                # 1) STT: E = beta*E - sgn_prev  (in place on G tile)
                for p, t in act:
                    sl = pcols(p)
                    nc.vector.scalar_tensor_tensor(G[:, sl], G[:, sl], BETA,
                                                   spk_slots[t - 1][:, sl],
                                                   ALU.mult, ALU.subtract)

                # 2) C-add on engines (uniform fast chains)
                for p, t in act:
                    sl = pcols(p)
                    eng = nc.vector if CADD[p] == 'dve' else nc.gpsimd
                    eng.tensor_tensor(G[:, sl], G[:, sl], C[:, sl], ALU.add)

                # 3) spike-gen: sgn = Sign(E) in +-1 fp8 (ACT)
                for p, t in act:
                    sl = pcols(p)
                    nc.scalar.activation(spk_slots[t][:, sl], G[:, sl],
                                         ACTF.Sign)

                # 4) fc2 for the step all pairs have finished
                tq = w - LAG[-1]
                if 1 <= tq <= T:
                    c2p = c2ps.tile([20, PB], F32, tag="c2p")
                    s = spk_slots[tq]
                    for j in range(NG):
                        nc.tensor.matmul(c2p[:], w2_t[j][:],
                                         s[:, j * PB:(j + 1) * PB],
                                         start=(j == 0), stop=(j == NG - 1))
                    cs = csp.tile([20, PB], F32, tag="cs", name="cs")
                    nc.scalar.copy(cs[:], c2p[:])
                    # PE transposes into 4 bank-aligned psum columns
                    tp = tps.tile([128, 4 * 512], F32, tag="tp", name="tp")
                    for bt in range(NBT):
                        nc.tensor.matmul(tp[:, bt * 512:bt * 512 + 20],
                                         cs[:, bt * 128:(bt + 1) * 128],
                                         idt[:], start=True, stop=True,
                                         is_transpose=True)
                    tp3 = tp[:].rearrange("p (b x) -> p b x", x=512)
                    hi = tp3[:, :, 0:OUT]
                    lo = tp3[:, :, OUT:2 * OUT]
                    # layer-2 LIF on [128, 40]
                    cur = slice((tq - 1) * NBT * OUT, tq * NBT * OUT)
                    if tq == 1:
                        pm, psk = z40[:], z40[:]
                    else:
                        prv = slice((tq - 2) * NBT * OUT,
                                    (tq - 1) * NBT * OUT)
                        pm, psk = rec_mem[:, prv], rec_spk[:, prv]
                    nc.vector.scalar_tensor_tensor(
                        l2u[:], pm, BETA, psk, ALU.mult, ALU.subtract)
                    nc.vector.tensor_tensor(
                        l2v[:].rearrange("p (b o) -> p b o", o=OUT),
                        l2u[:].rearrange("p (b o) -> p b o", o=OUT),
                        hi, ALU.add)
                    nc.vector.tensor_tensor(
                        rec_mem[:, cur].rearrange("p (b o) -> p b o", o=OUT),
                        l2v[:].rearrange("p (b o) -> p b o", o=OUT),
                        lo, ALU.add)
                    nc.vector.tensor_scalar(rec_spk[:, cur],
                                            rec_mem[:, cur], 1.0,
                                            None, ALU.is_gt)

            nc.sync.dma_start(ospk_d.ap(), rec_spk[:])
            nc.sync.dma_start(omem_d.ap(), rec_mem[:])
            if DBG_DUMP:
                nc.sync.dma_start(dC_d.ap(), C[:])
                nc.sync.dma_start(dG_d.ap(), G[:])
                nc.sync.dma_start(dS_d.ap(),
                                  spk_slots[DBG_DUMP][:])

    nc.compile()
    return nc


_NC_CACHE = None


def _prep(x, W1, b1, W2, b2):
    # fc1: w1t = [W1.T; b1-row], padded to 1024 hidden
    w1t = np.zeros((KA, HIDP), np.float32)
    w1t[:INP, :HID] = W1.T
    w1t[INP, :HID] = b1
    w1t[INP, HID] = BIG          # unit 1000: always fires (bias carrier)
    w1t[INP, HID + 1:] = -BIG    # other pads: never fire

    # fc2 rows (+-1 sgn everywhere: rows W2T/2, bias row b2 + sum(W2T)/2)
    R = np.zeros((HIDP, OUT), np.float64)
    W2T = W2.T.astype(np.float64)
    R[:HID] = 0.5 * W2T
    R[HID] = b2.astype(np.float64) + 0.5 * W2T.sum(axis=0)
    hi = R.astype(BF16NP)
    lo = (R - hi.astype(np.float64)).astype(BF16NP)
    w2p = np.zeros((HIDP, 20), BF16NP)
    w2p[:, 0:OUT] = hi
    w2p[:, OUT:2 * OUT] = lo

    xt = np.concatenate([x.T, np.ones((1, x.shape[0]), np.float32)], axis=0)
    return w1t, w2p, xt


def kernel(x, W1, b1, W2, b2):
    global _NC_CACHE
    x = np.ascontiguousarray(np.asarray(x, np.float32))
    W1 = np.asarray(W1, np.float32)
    b1 = np.asarray(b1, np.float32)
    W2 = np.asarray(W2, np.float32)
    b2 = np.asarray(b2, np.float32)

    w1t, w2p, xt = _prep(x, W1, b1, W2, b2)

    if _NC_CACHE is None:
        _NC_CACHE = _build_program()
    nc = _NC_CACHE

    in_maps = []
    for c in range(N_CORES):
        sl = slice(c * PB, (c + 1) * PB)
        in_maps.append({
            "xt": np.ascontiguousarray(xt[:, sl]),
            "w1t": w1t,
            "w2p": w2p,
            "idt": np.eye(20, dtype=np.float32),
        })

    res = run_bass_kernel_spmd(nc, in_maps, core_ids=list(range(N_CORES)))
    kernel.last_results = res

    ospk = np.empty((T, B, OUT), np.float32)
    omem = np.empty((T, B, OUT), np.float32)
    # rec[p, t*40 + bt*10 + o] -> [t, 128*bt+p, o]
    for c in range(N_CORES):
        sl = slice(c * PB, (c + 1) * PB)
        a = res.results[c]["ospk"].reshape(128, T, NBT, OUT)
        ospk[:, sl, :] = a.transpose(1, 2, 0, 3).reshape(T, PB, OUT)
        a = res.results[c]["omem"].reshape(128, T, NBT, OUT)
        omem[:, sl, :] = a.transpose(1, 2, 0, 3).reshape(T, PB, OUT)
    return ospk, omem
